# revision 12
# baseline (speedup 1.0000x reference)
"""GAT forward (batch of 8 graphs) on 8 Trainium2 NeuronCores.

Batch-sharded SPMD: core b computes graph b.

Per-core math (adj [N,N], afm [N,F_IN], W [F_IN,F_OUT], a1/a2 [F_OUT]):
  mb      = rowmax(adj)                      [N,1]
  A       = adj + mb * I                     (output 1)
  h       = afm @ W                          [N,64]
  s1      = h @ a1 ; s2 = h @ a2             [N]
  e[i,j]  = leakyrelu(s1[i]+s2[j], 0.2)
  mask    = A > 0
  P       = mask * exp(e)        (no rowmax subtraction needed: |e| <~ 10)
  d       = rowsum(P)            (via ones-column in the h matmul)
  x       = relu((P @ h) / d)                (output 2)

exp(leakyrelu(z)) == max(exp(z), exp(0.2*z)) since exp is monotone, so the
score pass is two ACT exps (with per-partition bias) + one elementwise max.

P @ h runs on the PE as  y^T[o,i] = sum_j h[j,o] * P^T[j,i]  with P^T produced
by PE transposes (P is computed row-major with i on partitions).  h is
augmented with a ones column so row 64 of y^T is the softmax denominator d.
"""

import numpy as np

B, N, F_IN, F_OUT = 8, 2048, 128, 64
ALPHA = 0.2
PB = 128                 # partition block
NB = N // PB             # 16 row/col blocks
FO1 = F_OUT + 1          # h augmented with a ones column
TG = 4                   # PE transposes batched per PSUM bank
NCORES = 8

_CACHE = {}


def _emit(tc, nc, adj, afm, W, a1, a2, xo, Ao):
    import concourse.bass as bass
    import concourse.mybir as mybir
    from concourse.masks import make_identity

    fp32 = mybir.dt.float32
    AX = mybir.AxisListType.X
    OP = mybir.AluOpType
    AF = mybir.ActivationFunctionType

    with (
        tc.tile_pool(name="const", bufs=1) as const,
        tc.tile_pool(name="work", bufs=3) as work,
        tc.tile_pool(name="adjp", bufs=3) as adjp,
        tc.tile_pool(name="ew", bufs=2) as ew,
        tc.tile_pool(name="ptsb", bufs=3) as ptsb,
        tc.tile_pool(name="small", bufs=4) as small,
        tc.tile_pool(name="dram", bufs=1, space="DRAM") as dram,
        tc.tile_pool(name="ps_t", bufs=2, space="PSUM") as ps_t,
        tc.tile_pool(name="ps_acc", bufs=2, space="PSUM") as ps_acc,
        tc.tile_pool(name="ps_y", bufs=2, space="PSUM") as ps_y,
    ):
        # ---------- constants ----------
        ident = const.tile([PB, PB], fp32)
        make_identity(nc, ident)
        W_sb = const.tile([F_IN, F_OUT], fp32)
        nc.sync.dma_start(out=W_sb, in_=W)
        a1bc = const.tile([PB, F_OUT], fp32)
        nc.gpsimd.dma_start(
            out=a1bc,
            in_=bass.AP(tensor=a1.tensor, offset=a1.offset, ap=[[0, PB]] + list(a1.ap)),
        )
        a2bc = const.tile([PB, F_OUT], fp32)
        nc.gpsimd.dma_start(
            out=a2bc,
            in_=bass.AP(tensor=a2.tensor, offset=a2.offset, ap=[[0, PB]] + list(a2.ap)),
        )
        ones_row = const.tile([1, PB], fp32)
        nc.vector.memset(ones_row, 1.0)

        h_aug = const.tile([PB, NB * FO1], fp32)   # 16 chunks of [h | 1]
        s1_all = const.tile([PB, NB], fp32)
        s2_all = const.tile([PB, NB], fp32)
        s1_02 = const.tile([PB, NB], fp32)
        S2bc = const.tile([PB, N], fp32)           # s2[j] broadcast over partitions

        # ---------- h = afm @ W, s1, s2 ----------
        import os
        setup_stage = int(os.environ.get("GAT_SETUP_STAGE", 99))
        nc_chunks = NB if setup_stage >= 2 else (1 if setup_stage == 1 else 0)
        for c in range(nc_chunks):
            afm_sb = work.tile([PB, F_IN], fp32, tag="afm")
            nc.sync.dma_start(out=afm_sb, in_=afm[c * PB:(c + 1) * PB, :])
            afmT = ps_t.tile([PB, PB], fp32, tag="pt")
            nc.tensor.transpose(afmT, afm_sb, ident)
            afmT_sb = work.tile([PB, F_IN], fp32, tag="afmT")
            nc.scalar.copy(afmT_sb, afmT)
            h_ps = ps_y.tile([PB, F_OUT], fp32, tag="y")
            nc.tensor.matmul(h_ps, lhsT=afmT_sb, rhs=W_sb, start=True, stop=True)
            hc = h_aug[:, c * FO1:c * FO1 + F_OUT]
            nc.scalar.copy(hc, h_ps)
            nc.vector.memset(h_aug[:, c * FO1 + F_OUT:(c + 1) * FO1], 1.0)
            scr = small.tile([PB, F_OUT], fp32, tag="scr")
            nc.vector.tensor_tensor(out=scr, in0=hc, in1=a1bc, op=OP.mult)
            nc.vector.tensor_reduce(s1_all[:, c:c + 1], scr, axis=AX, op=OP.add)
            scr2 = small.tile([PB, F_OUT], fp32, tag="scr")
            nc.vector.tensor_tensor(out=scr2, in0=hc, in1=a2bc, op=OP.mult)
            nc.vector.tensor_reduce(s2_all[:, c:c + 1], scr2, axis=AX, op=OP.add)
        if setup_stage >= 2:
            nc.vector.tensor_scalar_mul(s1_02, s1_all, ALPHA)
        else:
            nc.vector.memset(s1_all, 0.0)
            nc.vector.memset(s1_02, 0.0)
            nc.vector.memset(s2_all, 0.0)
            nc.vector.memset(h_aug, 1.0)

        # s2 -> row layout -> broadcast across partitions (DRAM bounce with a
        # partition-stride-0 read AP)
        if setup_stage >= 3:
            s2T_ps = ps_t.tile([NB, PB], fp32, tag="pt")
            nc.tensor.transpose(s2T_ps, s2_all, ident)
            s2T_sb = const.tile([NB, PB], fp32)
            nc.scalar.copy(s2T_sb, s2T_ps)
            s2d = dram.tile([NB, PB], fp32)
            nc.sync.dma_start(out=s2d, in_=s2T_sb)
            nc.sync.dma_start(
                out=S2bc,
                in_=bass.AP(tensor=s2d.tensor, offset=s2d.offset,
                            ap=[[0, PB], [1, N]]),
            )
        else:
            nc.vector.memset(S2bc, 0.0)

        # ---------- main loop over row blocks ----------
        import os
        nb_limit = int(os.environ.get("GAT_NB_LIMIT", NB))
        for bi in range(nb_limit):
            r0 = bi * PB
            adjb = adjp.tile([PB, N], fp32, tag="adj")
            nc.sync.dma_start(out=adjb, in_=adj[r0:r0 + PB, :])

            mb = small.tile([PB, 1], fp32, tag="mb")
            nc.vector.reduce_max(mb, adjb, axis=AX)
            mbI = small.tile([PB, PB], fp32, tag="mbI")
            nc.vector.tensor_scalar_mul(mbI, ident, mb)
            Adiag = small.tile([PB, PB], fp32, tag="Adiag")
            nc.vector.tensor_tensor(out=Adiag, in0=adjb[:, r0:r0 + PB], in1=mbI,
                                    op=OP.add)
            # A output: unchanged spans straight from adjb, diag block fixed.
            if r0 > 0:
                nc.sync.dma_start(out=Ao[r0:r0 + PB, 0:r0], in_=adjb[:, 0:r0])
            nc.sync.dma_start(out=Ao[r0:r0 + PB, r0:r0 + PB], in_=Adiag)
            if r0 + PB < N:
                nc.sync.dma_start(out=Ao[r0:r0 + PB, r0 + PB:N],
                                  in_=adjb[:, r0 + PB:N])

            # P = mask * exp(leakyrelu(s1[i]+s2[j]))
            e1 = ew.tile([PB, N], fp32, tag="e1")
            nc.scalar.activation(e1, S2bc, AF.Exp, bias=s1_all[:, bi:bi + 1],
                                 scale=1.0)
            e2 = ew.tile([PB, N], fp32, tag="e2")
            nc.scalar.activation(e2, S2bc, AF.Exp, bias=s1_02[:, bi:bi + 1],
                                 scale=ALPHA)
            nc.vector.tensor_tensor(out=e2, in0=e1, in1=e2, op=OP.max)
            p = ew.tile([PB, N], fp32, tag="p")
            nc.gpsimd.tensor_tensor(out=p, in0=e2, in1=adjb, op=OP.mult)
            maskd = small.tile([PB, PB], fp32, tag="maskd")
            nc.vector.tensor_tensor(out=maskd, in0=adjb[:, r0:r0 + PB], in1=mbI,
                                    op=OP.max)
            nc.vector.tensor_tensor(out=p[:, r0:r0 + PB], in0=e2[:, r0:r0 + PB],
                                    in1=maskd, op=OP.mult)

            # y^T[o,i] (+ d in row 64) = sum_j h_aug[j,:].T P^T[j,i]
            yt = ps_acc.tile([FO1, PB], fp32, tag="yt")
            for g in range(NB // TG):
                pt_ps = ps_t.tile([PB, TG * PB], fp32, tag="pt")
                for k in range(TG):
                    jc = g * TG + k
                    nc.tensor.matmul(pt_ps[:, k * PB:(k + 1) * PB],
                                     lhsT=p[:, jc * PB:(jc + 1) * PB], rhs=ident,
                                     is_transpose=True,
                                     start=(k == 0), stop=(k == TG - 1))
                pt_sb = ptsb.tile([PB, TG * PB], fp32, tag="ptsb")
                if g % 2 == 0:
                    nc.scalar.copy(pt_sb, pt_ps)
                else:
                    nc.vector.tensor_copy(pt_sb, pt_ps)
                for k in range(TG):
                    jc = g * TG + k
                    nc.tensor.matmul(yt, lhsT=h_aug[:, jc * FO1:(jc + 1) * FO1],
                                     rhs=pt_sb[:, k * PB:(k + 1) * PB],
                                     start=(jc == 0), stop=(jc == NB - 1))

            yt_sb = small.tile([FO1, PB], fp32, tag="yts")
            nc.scalar.copy(yt_sb, yt)
            y_ps = ps_y.tile([PB, FO1], fp32, tag="y")
            nc.tensor.transpose(y_ps, yt_sb, ident[:FO1, :FO1])
            invd = small.tile([PB, 1], fp32, tag="invd")
            nc.vector.reciprocal(invd, y_ps[:, F_OUT:F_OUT + 1])
            x_sb = small.tile([PB, F_OUT], fp32, tag="x")
            nc.scalar.activation(x_sb, y_ps[:, 0:F_OUT], AF.Relu, bias=0.0,
                                 scale=invd)
            nc.sync.dma_start(out=xo[r0:r0 + PB, :], in_=x_sb)


def build_nc():
    import concourse.bacc as bacc
    import concourse.mybir as mybir
    import concourse.tile as tile

    fp32 = mybir.dt.float32
    nc = bacc.Bacc("TRN2", target_bir_lowering=False, debug=False,
                   enable_asserts=False, num_devices=NCORES)
    adj = nc.dram_tensor("adj", [N, N], fp32, kind="ExternalInput").ap()
    afm = nc.dram_tensor("afm", [N, F_IN], fp32, kind="ExternalInput").ap()
    W = nc.dram_tensor("W", [F_IN, F_OUT], fp32, kind="ExternalInput").ap()
    a1 = nc.dram_tensor("a1", [F_OUT], fp32, kind="ExternalInput").ap()
    a2 = nc.dram_tensor("a2", [F_OUT], fp32, kind="ExternalInput").ap()
    xo = nc.dram_tensor("x_out", [N, F_OUT], fp32, kind="ExternalOutput").ap()
    Ao = nc.dram_tensor("A_out", [N, N], fp32, kind="ExternalOutput").ap()
    with tile.TileContext(nc) as tc:
        _emit(tc, nc, adj, afm, W, a1, a2, xo, Ao)
    nc.compile()
    return nc


def _get_nc():
    if "nc" not in _CACHE:
        _CACHE["nc"] = build_nc()
    return _CACHE["nc"]


def kernel(adjs, afms, TypeAtt, OrderAtt, AromAtt, ConjAtt, RingAtt, W, a1, a2,
           **_unused):
    from concourse.bass_utils import run_bass_kernel_spmd

    nc = _get_nc()
    adjs = np.ascontiguousarray(np.asarray(adjs, dtype=np.float32))
    afms = np.ascontiguousarray(np.asarray(afms, dtype=np.float32))
    W = np.ascontiguousarray(np.asarray(W, dtype=np.float32))
    a1 = np.ascontiguousarray(np.asarray(a1, dtype=np.float32))
    a2 = np.ascontiguousarray(np.asarray(a2, dtype=np.float32))
    in_maps = [
        {"adj": adjs[b], "afm": afms[b], "W": W, "a1": a1, "a2": a2}
        for b in range(NCORES)
    ]
    res = run_bass_kernel_spmd(nc, in_maps, core_ids=list(range(NCORES)))
    x = np.stack([res.results[b]["x_out"] for b in range(NCORES)])
    A = np.stack([res.results[b]["A_out"] for b in range(NCORES)])
    return (x, A)
